# revision 37
# baseline (speedup 1.0000x reference)
"""Trainium2 Bass kernel for nn_AttentionWithTime (B=4, N=2048, in_c=512,
head_c=64, H=8, expand_c=2048, time_c=256), 8-core SPMD.

Sharding: token-parallel. Core c handles batch b=c//2 and query rows
(c%2)*1024 .. +1024 of that batch. Each core computes K/V for its whole
batch (2x redundant) so no cross-core collectives are needed; the host
splits inputs and concatenates the 8 per-core [1024, 512] outputs.

Host-side folds (pure input math, done once in kernel()):
  - ln1 gamma/beta folded into qkv_w/qkv_b
  - attention scale (head_c^-0.5) folded into the q columns
  - v bias folded into merge_b (softmax rows sum to 1)
  - merge_w folded into wv per head: WM_h = wv_h @ merge_w_h * S_SCALE
    (fp8; the scale keeps the product in e4m3's normal range and rides
    on the softmax reciprocal). This removes the on-device merge matmul:
    out_h = (A_h @ Z_h) * recT with Z = ln1(x) @ WM.
  - ln2 gamma/beta folded into ff1_w/ff1_b

Device pipeline (per core):
  A: LN1 + bf16 transposes -> XT (bf16) + one wide fp8 cast -> XT8;
     q^T/k^T; Z for heads 0,1 as the dense PE tail.
  C: 8 software-pipelined units (u = head-pair*2 + query-tile). Unit u's
     kc loop interleaves, per key chunk: the 2-head row-packed S matmuls
     (K=64 at row groups 0-63/64-127, concurrent in the PE), the A@Z
     matmuls of unit u-2 (A-stationary fp8 DoubleRow, token-major out,
     evicted straight into X2 scaled by the softmax reciprocal), Z
     production per the ZPROD schedule (front-loaded into the ACT-bound
     units 0/1), unit u-1's softmax-denominator halving tree (contiguous
     bf16 DVE adds, spread across the loop so they never delay the A@Z
     evictions queued behind them on the DVE), and at kc15 unit u-1's
     4 tiny cross-partition denominator matmuls + reciprocal.
  D: LN2 (time concat) + bf16 transposes + FFN, overlapping the flush
     of units 6/7. gpsimd only ever runs DMA issues + partition
     broadcasts (mixing op families forces ~7us microcode reloads).
"""

import numpy as np

B, N, IN_C, HEAD_C, EXPAND_C, OUT_C, TIME_C, H = 4, 2048, 512, 64, 2048, 512, 256, 8
EPS = 1e-5
NCORES = 8
NTOK = N  # tokens per batch handled per core (keys)
NQ = N // 2  # query rows owned per core
P = 128
S_SCALE = 256.0  # host scale on WM = wv@merge_w before fp8 cast
F1_SCALE = 256.0  # host scale on ff1_w before fp8 cast (undone in gelu scale)

QUANT = "bf16"


def _emit(nc, tc, tens, dt_op):
    import concourse.bass as bass
    from concourse import mybir
    from concourse.masks import make_identity
    from concourse.bass import ts

    f32 = mybir.dt.float32
    f32r = mybir.dt.float32r
    dt8 = mybir.dt.float8e4
    AF = mybir.ActivationFunctionType
    ALU = mybir.AluOpType

    x_roll = tens["x_roll"][:]
    wqk_d = tens["wqk"][:]
    bqk_d = tens["bqk"][:]
    wm_d = tens["wm"][:]
    mb_d = tens["merge_b"][:]
    t_d = tens["t_vec"][:]
    tw_d = tens["time_w"][:]
    tb_d = tens["time_b"][:]
    f1w_d = tens["ff1_w"][:]
    f1b_d = tens["ff1_b"][:]
    f2w_d = tens["ff2_w"][:]
    f2b_d = tens["ff2_b"][:]
    out_d = tens["out"][:]

    KC = IN_C // P  # 4 feature chunks of x
    TCH = NTOK // P  # 16 token chunks per batch
    QCH = NQ // P  # 8 own-token chunks
    NQT = NQ // 512  # 2 query tiles of 512
    NU = H  # pipeline units: (head pair)*2 + qt
    CCH = (IN_C + TIME_C) // P  # 6
    ECH = EXPAND_C // P  # 16

    # ---- long-lived pools ----
    const = tc.alloc_tile_pool(name="const", bufs=1)
    persist = tc.alloc_tile_pool(name="persist", bufs=1)

    ident = const.tile([P, P], dt_op)
    make_identity(nc, ident)
    eps_t = const.tile([P, 1], f32)
    nc.vector.memset(eps_t, EPS)
    sca_t = const.tile([P, 1], dt_op)
    nc.vector.memset(sca_t, S_SCALE)  # rhs of denominator matmuls

    # bias tiles (DMAs deferred into phase A so the x stream goes first)
    mb_row = const.tile([1, OUT_C], f32)
    mb_bc = const.tile([P, OUT_C], f32)
    f2b_row = const.tile([1, OUT_C], f32)
    f2b_bc = const.tile([P, OUT_C], f32)
    bqk_sb = const.tile([P, 8], f32)
    f1b_sb = const.tile([P, 16], f32)

    XT8 = persist.tile([P, KC // 2, 2, NTOK], dt8)  # fp8 copy
    qT = persist.tile([P, KC, NQ], dt_op)  # q^T (own rows), feature-major
    kT = persist.tile([P, KC, NTOK], dt_op)  # k^T, feature-major
    X2 = persist.tile([P, QCH, OUT_C], f32)  # x + attn + merge_b'
    tt_bc = const.tile([P, TIME_C], f32)  # t@time_w + time_b, bcast over rows
    st_tt = const.tile([P, 6], f32)  # bn_stats of tt_bc (same for all chunks)
    nrmA = persist.tile([P, QCH, IN_C + TIME_C], dt_op)  # ln2 normalized rows
    mvs = persist.tile([P, QCH, 2], f32)
    rstd8 = persist.tile([P, QCH], f32)

    def emit_ln2_stats(tci, phL):
        """LN2 stats for one token chunk (emit as soon as its X2 row is
        final so the chain overlaps the tail of attention)."""
        st2 = phL.tile([P, 3, 6], f32, tag="st2")
        nc.vector.bn_stats(out=st2[:, 0, :], in_=X2[:, tci, 0:256])
        nc.vector.bn_stats(out=st2[:, 1, :], in_=X2[:, tci, 256:512])
        nc.vector.tensor_copy(out=st2[:, 2, :], in_=st_tt)
        nc.vector.bn_aggr(out=mvs[:, tci, :], in_=st2)

    def emit_ln2(qt, phL, stats_done=False):
        """LN2 stats + normalize for one query tile's 4 token chunks."""
        if not stats_done:
            for tc4 in range(4):
                emit_ln2_stats(qt * 4 + tc4, phL)
        nc.scalar.activation(
            out=rstd8[:, qt * 4 : qt * 4 + 4],
            in_=mvs[:, qt * 4 : qt * 4 + 4, 1], func=AF.Sqrt, bias=eps_t,
        )
        nc.vector.reciprocal(
            out=rstd8[:, qt * 4 : qt * 4 + 4],
            in_=rstd8[:, qt * 4 : qt * 4 + 4],
        )
        for tc4 in range(4):
            tci = qt * 4 + tc4
            nc.vector.tensor_scalar(
                out=nrmA[:, tci, 0:IN_C], in0=X2[:, tci, :],
                scalar1=mvs[:, tci, 0:1], scalar2=rstd8[:, tci : tci + 1],
                op0=ALU.subtract, op1=ALU.mult,
            )
            nc.vector.tensor_scalar(
                out=nrmA[:, tci, IN_C:], in0=tt_bc,
                scalar1=mvs[:, tci, 0:1], scalar2=rstd8[:, tci : tci + 1],
                op0=ALU.subtract, op1=ALU.mult,
            )

    with (
        tc.tile_pool(name="phw", bufs=3) as phw,
        tc.tile_pool(name="phv", bufs=6) as phv,
        tc.tile_pool(name="phd", bufs=2) as phd,
        tc.tile_pool(name="phc", bufs=2) as phc,
        tc.tile_pool(name="psV", bufs=2, space="PSUM") as psV,
    ):
        xtpool = tc.alloc_tile_pool(name="xtpool", bufs=1)
        XT = xtpool.tile([P, KC, NTOK], dt_op)  # ln1(x)^T, feature-major
        z_tiles = {}  # h -> fp8 DoubleRow-packed Z = ln1(x) @ WM_h

        def new_wm(h):
            wm_sb = phw.tile(
                [P, KC // 2, 2, IN_C], dt8, tag="wm", name=f"wm{h}"
            )
            nc.gpsimd.dma_start(out=wm_sb, in_=wm_d[:, h])
            return wm_sb

        def new_z(h):
            z_tiles[h] = phv.tile(
                [P, TCH // 2, 2, IN_C], dt8, tag="z", name=f"z{h}"
            )

        def z_step(h, wm_sb, tci, evict=None):
            """Two DR matmuls + one copy eviction: Z chunk tci of head h."""
            pv = psV.tile([P, IN_C], f32, tag="pv")
            for kc2 in range(KC // 2):
                nc.tensor.matmul(
                    pv,
                    XT8[:, kc2, :, ts(tci, P)],
                    wm_sb[:, kc2, :, :],
                    start=(kc2 == 0),
                    stop=(kc2 == KC // 2 - 1),
                    perf_mode=mybir.MatmulPerfMode.DoubleRow,
                )
            dst = z_tiles[h][:, tci // 2, tci % 2, :]
            if evict is None:
                nc.vector.tensor_copy(out=dst, in_=pv)
            else:
                evict.copy(out=dst, in_=pv)

        # ---- phase A ----
        with (
            tc.tile_pool(name="pha", bufs=4) as pha,
            tc.tile_pool(name="pha1", bufs=4) as pha1,
            tc.tile_pool(name="psA", bufs=2, space="PSUM") as psA,
            tc.tile_pool(name="psAq", bufs=2, space="PSUM") as psAq,
            tc.tile_pool(name="psAt", bufs=1, space="PSUM") as psAt,
        ):
            # prefetch qkv weights split across all three DMA queues,
            # then WM for heads 0/1 on the gpsimd queue
            wqk_sb = pha.tile([P, KC, 2 * IN_C], dt_op, bufs=1)
            for c, eng in enumerate([nc.gpsimd, nc.sync, nc.scalar, nc.gpsimd]):
                eng.dma_start(out=wqk_sb[:, c, :], in_=wqk_d[ts(c, P), :])
            wm0 = new_wm(0)
            wm1 = new_wm(1)

            dma_engines = [nc.sync, nc.scalar]
            for tti in range(4):  # 512-token groups
                for sub in range(4):
                    tci = tti * 4 + sub
                    x_t = pha.tile([P, IN_C], dt_op)
                    dma_engines[tci % 2].dma_start(
                        out=x_t, in_=x_roll[ts(tci, P), :]
                    )
                    st = pha1.tile([P, 6], f32)
                    nc.vector.bn_stats(out=st, in_=x_t)
                    mv = pha1.tile([P, 2], f32)
                    nc.vector.bn_aggr(out=mv, in_=st)
                    rstd = pha1.tile([P, 1], f32)
                    nc.scalar.activation(
                        out=rstd, in_=mv[:, 1:2], func=AF.Sqrt, bias=eps_t
                    )
                    nc.vector.reciprocal(out=rstd, in_=rstd)
                    xn = pha.tile([P, IN_C], dt_op)
                    nc.vector.tensor_scalar(
                        out=xn, in0=x_t, scalar1=mv[:, 0:1], scalar2=rstd,
                        op0=ALU.subtract, op1=ALU.mult,
                    )
                    for fc in range(KC):
                        pt = psA.tile([P, P], dt_op)
                        nc.tensor.transpose(pt, xn[:, ts(fc, P)], ident)
                        if fc % 2:
                            nc.scalar.copy(out=XT[:, fc, ts(tci, P)], in_=pt)
                        else:
                            nc.vector.tensor_copy(
                                out=XT[:, fc, ts(tci, P)], in_=pt
                            )
                if tti == 0:
                    # deferred small DMAs, behind the first token group's
                    # x tiles but ahead of their consumers
                    nc.sync.dma_start(
                        out=bqk_sb, in_=bqk_d.rearrange("(c p) -> p c", p=P)
                    )
                    nc.scalar.dma_start(
                        out=f1b_sb, in_=f1b_d.rearrange("(c p) -> p c", p=P)
                    )
                    nc.gpsimd.dma_start(out=mb_row, in_=mb_d[None, :])
                    nc.gpsimd.partition_broadcast(mb_bc, mb_row)
                    nc.gpsimd.dma_start(out=f2b_row, in_=f2b_d[None, :])
                    nc.gpsimd.partition_broadcast(f2b_bc, f2b_row)
                # q^T / k^T for this 512-token group. k chunks first
                # (and low feature-chunks first): head-pair 0's S matmuls
                # only need the fc=0 chunks, so they can start earliest.
                mcs = [4, 0, 5, 1, 6, 2, 7, 3] if tti < NQT else [4, 5, 6, 7]
                for mc in mcs:
                    pq = psAq.tile([P, 512], f32)
                    for kc in range(KC):
                        nc.tensor.matmul(
                            pq,
                            wqk_sb[:, kc, ts(mc, P)],
                            XT[:, kc, ts(tti, 512)],
                            start=(kc == 0),
                            stop=(kc == KC - 1),
                        )
                    dest = (
                        qT[:, mc, ts(tti, 512)]
                        if mc < 4
                        else kT[:, mc - 4, ts(tti, 512)]
                    )
                    nc.scalar.activation(
                        out=dest, in_=pq, func=AF.Identity,
                        bias=bqk_sb[:, mc : mc + 1],
                    )
                # fp8 copy of this token group's XT (one wide DVE cast)
                nc.vector.tensor_copy(
                    out=XT8[:, :, :, ts(tti, 512)].rearrange(
                        "p c j t -> p (c j) t"
                    ),
                    in_=XT[:, :, ts(tti, 512)],
                )
            # time embedding: tt = t @ time_w + time_b  -> broadcast tile
            tT = pha.tile([P, 2], f32, bufs=1)
            nc.sync.dma_start(out=tT, in_=t_d.rearrange("(c p) -> p c", p=P))
            tw_sb = pha.tile([P, 2, TIME_C], f32, bufs=1)
            nc.sync.dma_start(out=tw_sb, in_=tw_d.rearrange("(c p) m -> p c m", p=P))
            tb_sb = pha.tile([1, TIME_C], f32, bufs=1)
            nc.sync.dma_start(out=tb_sb, in_=tb_d[None, :])
            ps_tt = psAt.tile([1, TIME_C], f32)
            for c in range(2):
                nc.tensor.matmul(
                    ps_tt, tT[:, c : c + 1], tw_sb[:, c, :],
                    start=(c == 0), stop=(c == 1),
                )
            tt_row = pha.tile([1, TIME_C], f32, bufs=1)
            nc.vector.tensor_add(tt_row, ps_tt, tb_sb)
            nc.gpsimd.partition_broadcast(tt_bc, tt_row)
            nc.vector.bn_stats(out=st_tt, in_=tt_bc)

            # Z for heads 0 and 1: dense PE tail of phase A
            new_z(0)
            for tci in range(TCH):
                z_step(0, wm0, tci)
            new_z(1)
            for tci in range(TCH):
                z_step(1, wm1, tci)

        xtpool.release()

        # ---- phase C: software-pipelined attention units ----
        with (
            tc.tile_pool(name="psO", bufs=2, space="PSUM") as psO,
            tc.tile_pool(name="phs", bufs=3) as phs,
            tc.tile_pool(name="phsm", bufs=2) as phsm,
            tc.tile_pool(name="psS", bufs=2, space="PSUM") as psS,
        ):
            # init X2 accumulator: own rows of x + merge_b' (gpsimd adds)
            for tci in range(QCH):
                xo = phc.tile([P, OUT_C], dt_op, tag="xo")
                nc.sync.dma_start(out=xo, in_=x_roll[ts(tci, P), :])
                nc.vector.tensor_add(X2[:, tci, :], xo, mb_bc)

            units = {}  # u -> dict of tiles
            # Z production schedule: front-loaded into the ACT-bound units
            # 0/1 (no A@Z interleave there) to keep the PE dense
            ZPROD = {0: [2, 3], 1: [4], 2: [5], 4: [6], 5: [7]}

            def emit_denom(u):
                """Cross-partition denominator totals + reciprocal for unit
                u (sden ready since unit u's end; runs inside unit u+1)."""
                U = units[u]
                pT = psV.tile([P, 8], f32, tag="pv", name=f"psumT{u}")
                for tag, off in (("A", 0), ("B", 4)):
                    sden_r = U["sden" + tag]
                    for c in range(4):
                        nc.tensor.matmul(
                            pT[:, off + c : off + c + 1],
                            sden_r[:, ts(c, P)], sca_t,
                            start=True, stop=True,
                        )
                recT = U["recT"] = phsm.tile([P, 8], f32, tag="recT", name=f"recT{u}")
                nc.vector.reciprocal(out=recT, in_=pT)

            def tree_steps(u):
                """Softmax-denominator halving tree for unit u as 12 lazy
                steps of <=2048 elements (spread through unit u+1's kc loop
                so no single DVE burst delays the A@Z evictions behind it).
                Splits at multiples of 1024 keep (j, q) blocks aligned."""
                U = units[u]
                per_head = []
                for tag in ("A", "B"):
                    e = U["exp" + tag].rearrange("p c j q -> p (c j q)")
                    sa = phsm.tile([P, 2048], dt_op, tag="tra" + tag,
                                   name=f"tra{tag}{u}", bufs=1)
                    sb = phsm.tile([P, 2048], dt_op, tag="trb" + tag,
                                   name=f"trb{tag}{u}", bufs=1)
                    sden = U["sden" + tag] = phsm.tile(
                        [P, 512], dt_op, tag="sden" + tag,
                        name=f"sden{tag}{u}",
                    )

                    def mk(e=e, sa=sa, sb=sb, sden=sden):
                        def add(o, a, b):
                            with nc.allow_low_precision(reason="denom"):
                                nc.vector.tensor_add(o, a, b)
                        return [
                            lambda: add(sa, e[:, 0:2048], e[:, 4096:6144]),
                            lambda: add(sb, e[:, 2048:4096], e[:, 6144:8192]),
                            lambda: add(sa[:, 0:1024], sa[:, 0:1024],
                                        sa[:, 1024:2048]),
                            lambda: add(sb[:, 0:1024], sb[:, 0:1024],
                                        sb[:, 1024:2048]),
                            lambda: add(sa[:, 0:1024], sa[:, 0:1024],
                                        sb[:, 0:1024]),
                            lambda: add(sden, sa[:, 0:512], sa[:, 512:1024]),
                        ]

                    per_head.append(mk())
                return [per_head[h][l] for l in range(6) for h in range(2)]

            def az_groups(u):
                """Closure emitting A@Z group g (0..7) of unit u: 8 DR
                matmuls (A-stationary, token-major out) + eviction into X2
                scaled by the softmax reciprocal."""
                j, qt = u // 2, u % 2
                U = units[u]

                def emit_group(g):
                    head_b, tc4 = divmod(g, 4)
                    expS = U["expB" if head_b else "expA"]
                    z_t = z_tiles[2 * j + head_b]
                    po = psO.tile([P, OUT_C], f32, tag="po")
                    for kc2 in range(TCH // 2):
                        nc.tensor.matmul(
                            po,
                            expS[:, kc2, :, ts(tc4, P)],
                            z_t[:, kc2, :, :],
                            start=(kc2 == 0),
                            stop=(kc2 == TCH // 2 - 1),
                            perf_mode=mybir.MatmulPerfMode.DoubleRow,
                        )
                    tci = qt * 4 + tc4
                    s0 = 4 * head_b + tc4
                    # scale on ACT (frees the psO slot fast), add on DVE
                    tmp = phsm.tile([P, OUT_C], f32, tag="potmp",
                                    name=f"potmp{u}g{g}", bufs=2)
                    nc.scalar.activation(
                        out=tmp, in_=po, func=AF.Identity,
                        scale=U["recT"][:, s0 : s0 + 1],
                    )
                    nc.vector.tensor_add(X2[:, tci, :], tmp, X2[:, tci, :])

                return emit_group

            def emit_S(u):
                """S matmuls + exp for both heads of unit u, interleaved
                with A@Z of u-2, Z production for head u, denominators of
                u-1."""
                j, qt = u // 2, u % 2
                U = units[u] = {}
                expA = U["expA"] = phs.tile(
                    [P, TCH // 2, 2, 512], dt8, tag="expA", name=f"expA{u}"
                )
                expB = U["expB"] = phs.tile(
                    [P, TCH // 2, 2, 512], dt8, tag="expB", name=f"expB{u}"
                )
                zprod = []
                for h in ZPROD.get(u, ()):
                    wm_h = new_wm(h)
                    new_z(h)
                    zprod.append((h, wm_h))
                az = az_groups(u - 2) if u >= 2 else None
                tsteps = tree_steps(u - 1) if u >= 1 else []
                for kc2 in range(TCH // 2):
                    pssA = psS.tile([P, 2, 512], f32, tag="pss2")
                    pssB = psS.tile([P, 2, 512], f32, tag="pss2")
                    for jj in range(2):
                        kc = 2 * kc2 + jj
                        nc.tensor.matmul(
                            pssA[:, jj, :], kT[0:64, j, ts(kc, P)],
                            qT[0:64, j, ts(qt, 512)],
                            start=True, stop=True,
                        )
                        nc.tensor.matmul(
                            pssB[:, jj, :], kT[64:128, j, ts(kc, P)],
                            qT[64:128, j, ts(qt, 512)],
                            start=True, stop=True,
                        )
                    # one exp per head per key pair: both halves at once
                    nc.scalar.activation(
                        out=expA[:, kc2, :, :], in_=pssA, func=AF.Exp
                    )
                    nc.scalar.activation(
                        out=expB[:, kc2, :, :], in_=pssB, func=AF.Exp
                    )
                    if az is not None:
                        az(kc2)
                    if tsteps:
                        # steps 0-3 are the 2.6us level-1 adds: one per
                        # slot; the smaller steps pair up in slots 4-7
                        if kc2 < 4:
                            tsteps[kc2]()
                        else:
                            tsteps[2 * kc2 - 4]()
                            tsteps[2 * kc2 - 3]()
                    if kc2 == 7 and u >= 1:
                        emit_denom(u - 1)
                    for zi, (h, wm_h) in enumerate(zprod):
                        for jj in range(2):
                            z_step(h, wm_h, 2 * kc2 + jj,
                                   evict=nc.scalar
                                   if u < 2 and (zi + jj) % 2 else None)


            for u in range(NU):
                emit_S(u)

            # flush: A@Z of unit 6 first (its evictions must not queue
            # behind the exp-gated tree of unit 7 on the DVE), then the
            # tree + denominators of 7, LN2(qt0), A@Z of 7
            az = az_groups(NU - 2)
            for g in range(8):
                az(g)
                if g >= 4:  # X2 row (g-4) of qt0 is final after head B's add
                    emit_ln2_stats(g - 4, phd)
            for step in tree_steps(NU - 1):
                step()
            emit_denom(NU - 1)
            emit_ln2(0, phd, stats_done=True)
            az = az_groups(NU - 1)
            for g in range(8):
                az(g)
                if g >= 4:
                    emit_ln2_stats(4 + g - 4, phd)

        # ---- phase D: LN2 transposes + FFN + residual ----
        with (
            tc.tile_pool(name="phh", bufs=1) as phh,
            tc.tile_pool(name="phdw", bufs=1) as phdw,
            tc.tile_pool(name="psDt", bufs=3, space="PSUM") as psDt,
            tc.tile_pool(name="psDf", bufs=2, space="PSUM") as psDf,
        ):
            f1w_sb = phdw.tile([P, CCH // 2, 2, EXPAND_C], dt8)
            nc.gpsimd.dma_start(out=f1w_sb, in_=f1w_d)
            f2w_sb = phdw.tile([P, ECH, OUT_C], dt_op)
            nc.gpsimd.dma_start(
                out=f2w_sb, in_=f2w_d.rearrange("(c p) m -> p c m", p=P)
            )

            def emit_l2T(qt, act_only=False):
                # act_only: the DVE is busy with the attention flush when
                # qt0's transposes run, while the ACT queue is idle there
                l2T = phh.tile([P, CCH // 2, 2, 512], dt8, tag=f"l2T{qt}")
                for tc4 in range(4):
                    tci = qt * 4 + tc4
                    for fc in range(CCH):
                        pt2 = psDt.tile([P, P], dt_op)
                        nc.tensor.transpose(pt2, nrmA[:, tci, ts(fc, P)], ident)
                        dst = l2T[:, fc // 2, fc % 2, ts(tc4, P)]
                        if act_only or fc % 2:
                            nc.scalar.copy(out=dst, in_=pt2)
                        else:
                            nc.vector.tensor_copy(out=dst, in_=pt2)
                return l2T

            def emit_ff1(qt, l2T):
                hT = phh.tile([P, ECH, 512], dt_op, tag="hT")
                for mc in range(ECH):
                    pf = psDf.tile([P, 512], f32, tag="pf")
                    for kc in range(CCH // 2):
                        nc.tensor.matmul(
                            pf, f1w_sb[:, kc, :, ts(mc, P)], l2T[:, kc, :, :],
                            start=(kc == 0), stop=(kc == CCH // 2 - 1),
                            perf_mode=mybir.MatmulPerfMode.DoubleRow,
                        )
                    nc.scalar.activation(
                        out=hT[:, mc, :], in_=pf, func=AF.Gelu,
                        bias=f1b_sb[:, mc : mc + 1], scale=1.0 / F1_SCALE,
                    )
                return hT

            def emit_ff2(qt, hT):
                for tc4 in range(4):
                    tci = qt * 4 + tc4
                    xb = phd.tile([P, OUT_C], f32, tag="xb")
                    nc.vector.tensor_add(xb, X2[:, tci, :], f2b_bc)
                    pg = psDf.tile([P, OUT_C], f32, tag="pf")
                    for kc in range(ECH):
                        nc.tensor.matmul(
                            pg, hT[:, kc, ts(tc4, P)], f2w_sb[:, kc, :],
                            start=(kc == 0), stop=(kc == ECH - 1),
                        )
                    outt = phd.tile([P, OUT_C], f32, tag="outt")
                    nc.vector.tensor_add(outt, pg, xb)
                    nc.sync.dma_start(out=out_d[ts(tci, P), :], in_=outt)

            l2T0 = emit_l2T(0, act_only=True)
            emit_ln2(1, phd, stats_done=True)
            hT0 = emit_ff1(0, l2T0)
            l2T1 = emit_l2T(1)
            emit_ff2(0, hT0)
            hT1 = emit_ff1(1, l2T1)
            emit_ff2(1, hT1)

    persist.release()
    const.release()


def build_program(quant=QUANT):
    import concourse.bass as bass
    import concourse.tile as tile
    from concourse import mybir, bacc

    f32 = mybir.dt.float32
    dt_op = mybir.dt.bfloat16
    dt_w = dt_op

    nc = bacc.Bacc(None, target_bir_lowering=False)
    tens = {}

    def inp(name, shape, dt):
        tens[name] = nc.dram_tensor(name, list(shape), dt, kind="ExternalInput")

    inp("x_roll", (NTOK, IN_C), dt_w)
    inp("wqk", (IN_C, 2 * IN_C), dt_w)
    inp("bqk", (2 * IN_C,), f32)
    inp("wm", (P, H, IN_C // (2 * P), 2, IN_C), mybir.dt.float8e4)
    inp("merge_b", (OUT_C,), f32)
    inp("t_vec", (TIME_C,), f32)
    inp("time_w", (TIME_C, TIME_C), f32)
    inp("time_b", (TIME_C,), f32)
    inp("ff1_w", (P, (IN_C + TIME_C) // (2 * P), 2, EXPAND_C),
        mybir.dt.float8e4)
    inp("ff1_b", (EXPAND_C,), f32)
    inp("ff2_w", (EXPAND_C, OUT_C), dt_w)
    inp("ff2_b", (OUT_C,), f32)
    tens["out"] = nc.dram_tensor("out", [NQ, OUT_C], f32, kind="ExternalOutput")

    with tile.TileContext(nc) as tc:
        _emit(nc, tc, tens, dt_op)
    nc.finalize()
    return nc


def make_in_maps(x, t, ln1_g, ln1_b, qkv_w, qkv_b, merge_w, merge_b, time_w,
                 time_b, ln2_g, ln2_b, ff1_w, ff1_b, ff2_w, ff2_b, quant=QUANT):
    import ml_dtypes

    f = np.float32
    npdt = ml_dtypes.bfloat16

    x = np.asarray(x, f)
    qkv_w = np.asarray(qkv_w, f)
    qkv_b = np.asarray(qkv_b, f)
    # fold ln1 affine into qkv
    qkv_wf = np.asarray(ln1_g, f)[:, None] * qkv_w
    qkv_bf = qkv_b + np.asarray(ln1_b, f) @ qkv_w
    scale = HEAD_C**-0.5
    qkv_wf[:, : H * HEAD_C] *= scale
    qkv_bf[: H * HEAD_C] *= scale
    wqk = qkv_wf[:, : 2 * H * HEAD_C]
    bqk = qkv_bf[: 2 * H * HEAD_C]
    wv = qkv_wf[:, 2 * H * HEAD_C :]
    bv = qkv_bf[2 * H * HEAD_C :]
    merge_w = np.asarray(merge_w, f)
    # fold v bias into merge_b (softmax rows sum to 1)
    merge_bf = np.asarray(merge_b, f) + bv @ merge_w
    # fold merge_w into wv per head (scaled into fp8's normal range; the
    # 1/S_SCALE rides on the softmax reciprocal via the sca_t matmul rhs)
    wm = np.empty((IN_C, H * IN_C), f)
    for h in range(H):
        wm[:, h * IN_C : (h + 1) * IN_C] = (
            wv[:, h * IN_C : (h + 1) * IN_C]
            @ merge_w[h * IN_C : (h + 1) * IN_C, :]
        ) * S_SCALE
    # pack to the device layout [p, h, c, j, m]: row index = c*256+j*128+p,
    # so each head's slab is one contiguous 2KB line per partition
    wm = np.ascontiguousarray(
        wm.reshape(2, 2, P, H, IN_C).transpose(2, 3, 0, 1, 4)
    )
    # fold ln2 affine into ff1; scale + pack for fp8 DoubleRow
    # (row index = cc2*256 + jj*128 + p -> [p, cc2, jj, m], contiguous DMA)
    ff1_wf = np.asarray(ln2_g, f)[:, None] * np.asarray(ff1_w, f)
    ff1_bf = np.asarray(ff1_b, f) + np.asarray(ln2_b, f) @ np.asarray(ff1_w, f)
    ff1_wf = np.ascontiguousarray(
        (ff1_wf * F1_SCALE).reshape(3, 2, P, EXPAND_C).transpose(2, 0, 1, 3)
    )

    shared = {
        "wqk": wqk.astype(npdt),
        "bqk": bqk.astype(f),
        "wm": wm.astype(ml_dtypes.float8_e4m3),
        "merge_b": merge_bf.astype(f),
        "time_w": np.asarray(time_w, f),
        "time_b": np.asarray(time_b, f),
        "ff1_w": ff1_wf.astype(ml_dtypes.float8_e4m3),
        "ff1_b": ff1_bf.astype(f),
        "ff2_w": np.asarray(ff2_w, f).astype(npdt),
        "ff2_b": np.asarray(ff2_b, f),
    }
    in_maps = []
    for c in range(NCORES):
        b, half = divmod(c, 2)
        xb = x[b]
        x_roll = np.concatenate([xb[half * NQ :], xb[: half * NQ]], axis=0)
        m = dict(shared)
        m["x_roll"] = np.ascontiguousarray(x_roll).astype(npdt)
        m["t_vec"] = np.asarray(t, f)[b]
        in_maps.append(m)
    return in_maps


_CACHE = {}


def kernel(**inputs):
    from concourse.bass_utils import run_bass_kernel_spmd

    if "nc" not in _CACHE:
        _CACHE["nc"] = build_program(QUANT)
    nc = _CACHE["nc"]
    in_maps = make_in_maps(**inputs, quant=QUANT)
    res = run_bass_kernel_spmd(nc, in_maps, core_ids=list(range(NCORES)))
    out = np.stack([res.results[c]["out"] for c in range(NCORES)], axis=0)
    return out.reshape(B, N, OUT_C)


# revision 38
# speedup vs baseline: 1.2324x; 1.2324x over previous
"""Trainium2 Bass kernel for nn_AttentionWithTime (B=4, N=2048, in_c=512,
head_c=64, H=8, expand_c=2048, time_c=256), 8-core SPMD.

Sharding: token-parallel. Core c handles batch b=c//2 and query rows
(c%2)*1024 .. +1024 of that batch. Each core computes K/V for its whole
batch (2x redundant) so no cross-core collectives are needed; the host
splits inputs and concatenates the 8 per-core [1024, 512] outputs.

Host-side folds (pure input math, done once in kernel()):
  - ln1 gamma/beta folded into qkv_w/qkv_b
  - attention scale (head_c^-0.5) folded into the q columns
  - v bias folded into merge_b (softmax rows sum to 1)
  - merge_w folded into wv per head: WM_h = wv_h @ merge_w_h * S_SCALE
    (fp8; the scale keeps the product in e4m3's normal range and rides
    on the softmax reciprocal). This removes the on-device merge matmul:
    out_h = (A_h @ Z_h) * recT with Z = ln1(x) @ WM.
  - ln2 gamma/beta folded into ff1_w/ff1_b

Device pipeline (per core):
  A: LN1 + bf16 transposes -> XT (bf16) + one wide fp8 cast -> XT8;
     q^T/k^T; Z for heads 0,1 as the dense PE tail.
  C: 8 software-pipelined units (u = head-pair*2 + query-tile). Unit u's
     kc loop interleaves, per key chunk: the 2-head row-packed S matmuls
     (K=64 at row groups 0-63/64-127, concurrent in the PE), the A@Z
     matmuls of unit u-2 (A-stationary fp8 DoubleRow, token-major out,
     evicted straight into X2 scaled by the softmax reciprocal), Z
     production per the ZPROD schedule (front-loaded into the ACT-bound
     units 0/1), unit u-1's softmax-denominator halving tree (contiguous
     bf16 DVE adds, spread across the loop so they never delay the A@Z
     evictions queued behind them on the DVE), and at kc15 unit u-1's
     4 tiny cross-partition denominator matmuls + reciprocal.
  D: LN2 (time concat) + bf16 transposes + FFN, overlapping the flush
     of units 6/7. gpsimd only ever runs DMA issues + partition
     broadcasts (mixing op families forces ~7us microcode reloads).
"""

import numpy as np

B, N, IN_C, HEAD_C, EXPAND_C, OUT_C, TIME_C, H = 4, 2048, 512, 64, 2048, 512, 256, 8
EPS = 1e-5
NCORES = 8
NTOK = N  # tokens per batch handled per core (keys)
NQ = N // 2  # query rows owned per core
P = 128
S_SCALE = 256.0  # host scale on WM = wv@merge_w before fp8 cast
F1_SCALE = 256.0  # host scale on ff1_w before fp8 cast (undone in gelu scale)
F2_SCALE = 256.0  # host scale on ff2_w before fp8 cast (undone at eviction)

QUANT = "bf16"


def _emit(nc, tc, tens, dt_op):
    import concourse.bass as bass
    from concourse import mybir
    from concourse.masks import make_identity
    from concourse.bass import ts

    f32 = mybir.dt.float32
    f32r = mybir.dt.float32r
    dt8 = mybir.dt.float8e4
    AF = mybir.ActivationFunctionType
    ALU = mybir.AluOpType

    x_roll = tens["x_roll"][:]
    wqk_d = tens["wqk"][:]
    bqk_d = tens["bqk"][:]
    wm_d = tens["wm"][:]
    mb_d = tens["merge_b"][:]
    t_d = tens["t_vec"][:]
    tw_d = tens["time_w"][:]
    tb_d = tens["time_b"][:]
    f1w_d = tens["ff1_w"][:]
    f1b_d = tens["ff1_b"][:]
    f2w_d = tens["ff2_w"][:]
    f2b_d = tens["ff2_b"][:]
    out_d = tens["out"][:]

    KC = IN_C // P  # 4 feature chunks of x
    TCH = NTOK // P  # 16 token chunks per batch
    QCH = NQ // P  # 8 own-token chunks
    NQT = NQ // 512  # 2 query tiles of 512
    NU = H  # pipeline units: (head pair)*2 + qt
    CCH = (IN_C + TIME_C) // P  # 6
    ECH = EXPAND_C // P  # 16

    # ---- long-lived pools ----
    const = tc.alloc_tile_pool(name="const", bufs=1)
    persist = tc.alloc_tile_pool(name="persist", bufs=1)

    ident = const.tile([P, P], dt_op)
    make_identity(nc, ident)
    eps_t = const.tile([P, 1], f32)
    nc.vector.memset(eps_t, EPS)
    sca_t = const.tile([P, 1], dt_op)
    nc.vector.memset(sca_t, S_SCALE)  # rhs of denominator matmuls
    s2i_t = const.tile([P, 1], f32)
    nc.vector.memset(s2i_t, 1.0 / F2_SCALE)

    # bias tiles (DMAs deferred into phase A so the x stream goes first)
    mb_row = const.tile([1, OUT_C], f32)
    mb_bc = const.tile([P, OUT_C], f32)
    f2b_row = const.tile([1, OUT_C], f32)
    f2b_bc = const.tile([P, OUT_C], f32)
    bqk_sb = const.tile([P, 8], f32)
    f1b_sb = const.tile([P, 16], f32)

    XT8 = persist.tile([P, KC // 2, 2, NTOK], dt8)  # fp8 copy
    qT = persist.tile([P, KC, NQ], dt_op)  # q^T (own rows), feature-major
    kT = persist.tile([P, KC, NTOK], dt_op)  # k^T, feature-major
    X2 = persist.tile([P, QCH, OUT_C], f32)  # x + attn + merge_b'
    tt_bc = const.tile([P, TIME_C], f32)  # t@time_w + time_b, bcast over rows
    st_tt = const.tile([P, 6], f32)  # bn_stats of tt_bc (same for all chunks)
    nrmA = persist.tile([P, QCH, IN_C + TIME_C], dt_op)  # ln2 normalized rows
    mvs = persist.tile([P, QCH, 2], f32)
    rstd8 = persist.tile([P, QCH], f32)

    def emit_ln2_stats(tci, phL):
        """LN2 stats for one token chunk (emit as soon as its X2 row is
        final so the chain overlaps the tail of attention)."""
        st2 = phL.tile([P, 3, 6], f32, tag="st2")
        nc.vector.bn_stats(out=st2[:, 0, :], in_=X2[:, tci, 0:256])
        nc.vector.bn_stats(out=st2[:, 1, :], in_=X2[:, tci, 256:512])
        nc.vector.tensor_copy(out=st2[:, 2, :], in_=st_tt)
        nc.vector.bn_aggr(out=mvs[:, tci, :], in_=st2)

    def emit_ln2(qt, phL, stats_done=False):
        """LN2 stats + normalize for one query tile's 4 token chunks."""
        if not stats_done:
            for tc4 in range(4):
                emit_ln2_stats(qt * 4 + tc4, phL)
        nc.scalar.activation(
            out=rstd8[:, qt * 4 : qt * 4 + 4],
            in_=mvs[:, qt * 4 : qt * 4 + 4, 1], func=AF.Sqrt, bias=eps_t,
        )
        nc.vector.reciprocal(
            out=rstd8[:, qt * 4 : qt * 4 + 4],
            in_=rstd8[:, qt * 4 : qt * 4 + 4],
        )
        for tc4 in range(4):
            tci = qt * 4 + tc4
            nc.vector.tensor_scalar(
                out=nrmA[:, tci, 0:IN_C], in0=X2[:, tci, :],
                scalar1=mvs[:, tci, 0:1], scalar2=rstd8[:, tci : tci + 1],
                op0=ALU.subtract, op1=ALU.mult,
            )
            nc.vector.tensor_scalar(
                out=nrmA[:, tci, IN_C:], in0=tt_bc,
                scalar1=mvs[:, tci, 0:1], scalar2=rstd8[:, tci : tci + 1],
                op0=ALU.subtract, op1=ALU.mult,
            )

    with (
        tc.tile_pool(name="phw", bufs=3) as phw,
        tc.tile_pool(name="phv", bufs=6) as phv,
        tc.tile_pool(name="phd", bufs=2) as phd,
        tc.tile_pool(name="phc", bufs=2) as phc,
        tc.tile_pool(name="psV", bufs=2, space="PSUM") as psV,
    ):
        xtpool = tc.alloc_tile_pool(name="xtpool", bufs=1)
        XT = xtpool.tile([P, KC, NTOK], dt_op)  # ln1(x)^T, feature-major
        z_tiles = {}  # h -> fp8 DoubleRow-packed Z = ln1(x) @ WM_h

        def new_wm(h):
            wm_sb = phw.tile(
                [P, KC // 2, 2, IN_C], dt8, tag="wm", name=f"wm{h}"
            )
            nc.gpsimd.dma_start(out=wm_sb, in_=wm_d[:, h])
            return wm_sb

        def new_z(h):
            z_tiles[h] = phv.tile(
                [P, TCH // 2, 2, IN_C], dt8, tag="z", name=f"z{h}"
            )

        def z_step(h, wm_sb, tci, evict=None):
            """Two DR matmuls + one copy eviction: Z chunk tci of head h."""
            pv = psV.tile([P, IN_C], f32, tag="pv")
            for kc2 in range(KC // 2):
                nc.tensor.matmul(
                    pv,
                    XT8[:, kc2, :, ts(tci, P)],
                    wm_sb[:, kc2, :, :],
                    start=(kc2 == 0),
                    stop=(kc2 == KC // 2 - 1),
                    perf_mode=mybir.MatmulPerfMode.DoubleRow,
                )
            dst = z_tiles[h][:, tci // 2, tci % 2, :]
            if evict is None:
                nc.vector.tensor_copy(out=dst, in_=pv)
            else:
                evict.copy(out=dst, in_=pv)

        # ---- phase A ----
        with (
            tc.tile_pool(name="pha", bufs=4) as pha,
            tc.tile_pool(name="pha1", bufs=4) as pha1,
            tc.tile_pool(name="psA", bufs=2, space="PSUM") as psA,
            tc.tile_pool(name="psAq", bufs=2, space="PSUM") as psAq,
            tc.tile_pool(name="psAt", bufs=1, space="PSUM") as psAt,
        ):
            # prefetch qkv weights split across all three DMA queues,
            # then WM for heads 0/1 on the gpsimd queue
            wqk_sb = pha.tile([P, KC, 2 * IN_C], dt_op, bufs=1)
            for c, eng in enumerate([nc.gpsimd, nc.sync, nc.scalar, nc.gpsimd]):
                eng.dma_start(out=wqk_sb[:, c, :], in_=wqk_d[ts(c, P), :])
            wm0 = new_wm(0)
            wm1 = new_wm(1)

            dma_engines = [nc.sync, nc.scalar]
            for tti in range(4):  # 512-token groups
                for sub in range(4):
                    tci = tti * 4 + sub
                    x_t = pha.tile([P, IN_C], dt_op)
                    dma_engines[tci % 2].dma_start(
                        out=x_t, in_=x_roll[ts(tci, P), :]
                    )
                    st = pha1.tile([P, 6], f32)
                    nc.vector.bn_stats(out=st, in_=x_t)
                    mv = pha1.tile([P, 2], f32)
                    nc.vector.bn_aggr(out=mv, in_=st)
                    rstd = pha1.tile([P, 1], f32)
                    nc.scalar.activation(
                        out=rstd, in_=mv[:, 1:2], func=AF.Sqrt, bias=eps_t
                    )
                    nc.vector.reciprocal(out=rstd, in_=rstd)
                    xn = pha.tile([P, IN_C], dt_op)
                    nc.vector.tensor_scalar(
                        out=xn, in0=x_t, scalar1=mv[:, 0:1], scalar2=rstd,
                        op0=ALU.subtract, op1=ALU.mult,
                    )
                    for fc in range(KC):
                        pt = psA.tile([P, P], dt_op)
                        nc.tensor.transpose(pt, xn[:, ts(fc, P)], ident)
                        if fc % 2:
                            nc.scalar.copy(out=XT[:, fc, ts(tci, P)], in_=pt)
                        else:
                            nc.vector.tensor_copy(
                                out=XT[:, fc, ts(tci, P)], in_=pt
                            )
                if tti == 0:
                    # deferred small DMAs, behind the first token group's
                    # x tiles but ahead of their consumers
                    nc.sync.dma_start(
                        out=bqk_sb, in_=bqk_d.rearrange("(c p) -> p c", p=P)
                    )
                    nc.scalar.dma_start(
                        out=f1b_sb, in_=f1b_d.rearrange("(c p) -> p c", p=P)
                    )
                    nc.gpsimd.dma_start(out=mb_row, in_=mb_d[None, :])
                    nc.gpsimd.partition_broadcast(mb_bc, mb_row)
                    nc.gpsimd.dma_start(out=f2b_row, in_=f2b_d[None, :])
                    nc.gpsimd.partition_broadcast(f2b_bc, f2b_row)
                # q^T / k^T for this 512-token group. k chunks first
                # (and low feature-chunks first): head-pair 0's S matmuls
                # only need the fc=0 chunks, so they can start earliest.
                mcs = [4, 0, 5, 1, 6, 2, 7, 3] if tti < NQT else [4, 5, 6, 7]
                for mc in mcs:
                    pq = psAq.tile([P, 512], f32)
                    for kc in range(KC):
                        nc.tensor.matmul(
                            pq,
                            wqk_sb[:, kc, ts(mc, P)],
                            XT[:, kc, ts(tti, 512)],
                            start=(kc == 0),
                            stop=(kc == KC - 1),
                        )
                    dest = (
                        qT[:, mc, ts(tti, 512)]
                        if mc < 4
                        else kT[:, mc - 4, ts(tti, 512)]
                    )
                    nc.scalar.activation(
                        out=dest, in_=pq, func=AF.Identity,
                        bias=bqk_sb[:, mc : mc + 1],
                    )
                # fp8 copy of this token group's XT (one wide DVE cast)
                nc.vector.tensor_copy(
                    out=XT8[:, :, :, ts(tti, 512)].rearrange(
                        "p c j t -> p (c j) t"
                    ),
                    in_=XT[:, :, ts(tti, 512)],
                )
            # time embedding: tt = t @ time_w + time_b  -> broadcast tile
            tT = pha.tile([P, 2], f32, bufs=1)
            nc.sync.dma_start(out=tT, in_=t_d.rearrange("(c p) -> p c", p=P))
            tw_sb = pha.tile([P, 2, TIME_C], f32, bufs=1)
            nc.sync.dma_start(out=tw_sb, in_=tw_d.rearrange("(c p) m -> p c m", p=P))
            tb_sb = pha.tile([1, TIME_C], f32, bufs=1)
            nc.sync.dma_start(out=tb_sb, in_=tb_d[None, :])
            ps_tt = psAt.tile([1, TIME_C], f32)
            for c in range(2):
                nc.tensor.matmul(
                    ps_tt, tT[:, c : c + 1], tw_sb[:, c, :],
                    start=(c == 0), stop=(c == 1),
                )
            tt_row = pha.tile([1, TIME_C], f32, bufs=1)
            nc.vector.tensor_add(tt_row, ps_tt, tb_sb)
            nc.gpsimd.partition_broadcast(tt_bc, tt_row)
            nc.vector.bn_stats(out=st_tt, in_=tt_bc)

            # Z for heads 0 and 1: dense PE tail of phase A
            new_z(0)
            for tci in range(TCH):
                z_step(0, wm0, tci)
            new_z(1)
            for tci in range(TCH):
                z_step(1, wm1, tci)

        xtpool.release()

        # ---- phase C: software-pipelined attention units ----
        with (
            tc.tile_pool(name="psO", bufs=2, space="PSUM") as psO,
            tc.tile_pool(name="phs", bufs=3) as phs,
            tc.tile_pool(name="phsm", bufs=2) as phsm,
            tc.tile_pool(name="psS", bufs=2, space="PSUM") as psS,
        ):
            # init X2 accumulator: own rows of x + merge_b' (gpsimd adds)
            for tci in range(QCH):
                xo = phc.tile([P, OUT_C], dt_op, tag="xo")
                nc.sync.dma_start(out=xo, in_=x_roll[ts(tci, P), :])
                nc.vector.tensor_add(X2[:, tci, :], xo, mb_bc)

            units = {}  # u -> dict of tiles
            # Z production schedule: front-loaded into the ACT-bound units
            # 0/1 (no A@Z interleave there) to keep the PE dense
            ZPROD = {0: [2, 3], 1: [4], 2: [5], 4: [6], 5: [7]}

            def emit_denom(u):
                """Cross-partition denominator totals + reciprocal for unit
                u (sden ready since unit u's end; runs inside unit u+1)."""
                U = units[u]
                pT = psV.tile([P, 8], f32, tag="pv", name=f"psumT{u}")
                for tag, off in (("A", 0), ("B", 4)):
                    sden_r = U["sden" + tag]
                    for c in range(4):
                        nc.tensor.matmul(
                            pT[:, off + c : off + c + 1],
                            sden_r[:, ts(c, P)], sca_t,
                            start=True, stop=True,
                        )
                recT = U["recT"] = phsm.tile([P, 8], f32, tag="recT", name=f"recT{u}")
                nc.vector.reciprocal(out=recT, in_=pT)

            def tree_steps(u):
                """Softmax-denominator halving tree for unit u as 12 lazy
                steps of <=2048 elements (spread through unit u+1's kc loop
                so no single DVE burst delays the A@Z evictions behind it).
                Splits at multiples of 1024 keep (j, q) blocks aligned."""
                U = units[u]
                per_head = []
                for tag in ("A", "B"):
                    e = U["exp" + tag].rearrange("p c j q -> p (c j q)")
                    sa = phsm.tile([P, 2048], dt_op, tag="tra" + tag,
                                   name=f"tra{tag}{u}", bufs=1)
                    sb = phsm.tile([P, 2048], dt_op, tag="trb" + tag,
                                   name=f"trb{tag}{u}", bufs=1)
                    sden = U["sden" + tag] = phsm.tile(
                        [P, 512], dt_op, tag="sden" + tag,
                        name=f"sden{tag}{u}",
                    )

                    def mk(e=e, sa=sa, sb=sb, sden=sden):
                        def add(o, a, b):
                            with nc.allow_low_precision(reason="denom"):
                                nc.vector.tensor_add(o, a, b)
                        return [
                            lambda: add(sa, e[:, 0:2048], e[:, 4096:6144]),
                            lambda: add(sb, e[:, 2048:4096], e[:, 6144:8192]),
                            lambda: add(sa[:, 0:1024], sa[:, 0:1024],
                                        sa[:, 1024:2048]),
                            lambda: add(sb[:, 0:1024], sb[:, 0:1024],
                                        sb[:, 1024:2048]),
                            lambda: add(sa[:, 0:1024], sa[:, 0:1024],
                                        sb[:, 0:1024]),
                            lambda: add(sden, sa[:, 0:512], sa[:, 512:1024]),
                        ]

                    per_head.append(mk())
                return [per_head[h][l] for l in range(6) for h in range(2)]

            def az_groups(u):
                """Closure emitting A@Z group g (0..7) of unit u: 8 DR
                matmuls (A-stationary, token-major out) + eviction into X2
                scaled by the softmax reciprocal."""
                j, qt = u // 2, u % 2
                U = units[u]

                def emit_group(g):
                    head_b, tc4 = divmod(g, 4)
                    expS = U["expB" if head_b else "expA"]
                    z_t = z_tiles[2 * j + head_b]
                    po = psO.tile([P, OUT_C], f32, tag="po")
                    for kc2 in range(TCH // 2):
                        nc.tensor.matmul(
                            po,
                            expS[:, kc2, :, ts(tc4, P)],
                            z_t[:, kc2, :, :],
                            start=(kc2 == 0),
                            stop=(kc2 == TCH // 2 - 1),
                            perf_mode=mybir.MatmulPerfMode.DoubleRow,
                        )
                    tci = qt * 4 + tc4
                    s0 = 4 * head_b + tc4
                    # scale on ACT (frees the psO slot fast), add on DVE
                    tmp = phsm.tile([P, OUT_C], f32, tag="potmp",
                                    name=f"potmp{u}g{g}", bufs=2)
                    nc.scalar.activation(
                        out=tmp, in_=po, func=AF.Identity,
                        scale=U["recT"][:, s0 : s0 + 1],
                    )
                    nc.vector.tensor_add(X2[:, tci, :], tmp, X2[:, tci, :])

                return emit_group

            def emit_S(u):
                """S matmuls + exp for both heads of unit u, interleaved
                with A@Z of u-2, Z production for head u, denominators of
                u-1."""
                j, qt = u // 2, u % 2
                U = units[u] = {}
                expA = U["expA"] = phs.tile(
                    [P, TCH // 2, 2, 512], dt8, tag="expA", name=f"expA{u}"
                )
                expB = U["expB"] = phs.tile(
                    [P, TCH // 2, 2, 512], dt8, tag="expB", name=f"expB{u}"
                )
                zprod = []
                for h in ZPROD.get(u, ()):
                    wm_h = new_wm(h)
                    new_z(h)
                    zprod.append((h, wm_h))
                az = az_groups(u - 2) if u >= 2 else None
                tsteps = tree_steps(u - 1) if u >= 1 else []
                for kc2 in range(TCH // 2):
                    pssA = psS.tile([P, 2, 512], f32, tag="pss2")
                    pssB = psS.tile([P, 2, 512], f32, tag="pss2")
                    for jj in range(2):
                        kc = 2 * kc2 + jj
                        nc.tensor.matmul(
                            pssA[:, jj, :], kT[0:64, j, ts(kc, P)],
                            qT[0:64, j, ts(qt, 512)],
                            start=True, stop=True,
                        )
                        nc.tensor.matmul(
                            pssB[:, jj, :], kT[64:128, j, ts(kc, P)],
                            qT[64:128, j, ts(qt, 512)],
                            start=True, stop=True,
                        )
                    # one exp per head per key pair: both halves at once
                    nc.scalar.activation(
                        out=expA[:, kc2, :, :], in_=pssA, func=AF.Exp
                    )
                    nc.scalar.activation(
                        out=expB[:, kc2, :, :], in_=pssB, func=AF.Exp
                    )
                    if az is not None:
                        az(kc2)
                    if tsteps:
                        # steps 0-3 are the 2.6us level-1 adds: one per
                        # slot; the smaller steps pair up in slots 4-7
                        if kc2 < 4:
                            tsteps[kc2]()
                        else:
                            tsteps[2 * kc2 - 4]()
                            tsteps[2 * kc2 - 3]()
                    if kc2 == 7 and u >= 1:
                        emit_denom(u - 1)
                    for zi, (h, wm_h) in enumerate(zprod):
                        for jj in range(2):
                            z_step(h, wm_h, 2 * kc2 + jj,
                                   evict=nc.scalar
                                   if u < 2 and (zi + jj) % 2 else None)


            for u in range(NU):
                emit_S(u)

            # flush: A@Z of unit 6 first (its evictions must not queue
            # behind the exp-gated tree of unit 7 on the DVE), then the
            # tree + denominators of 7, LN2(qt0), A@Z of 7
            az = az_groups(NU - 2)
            for g in range(8):
                az(g)
                if g >= 4:  # X2 row (g-4) of qt0 is final after head B's add
                    emit_ln2_stats(g - 4, phd)
            for step in tree_steps(NU - 1):
                step()
            emit_denom(NU - 1)
            emit_ln2(0, phd, stats_done=True)
            az = az_groups(NU - 1)
            for g in range(8):
                az(g)
                if g >= 4:
                    emit_ln2_stats(4 + g - 4, phd)

        # ---- phase D: LN2 transposes + FFN + residual ----
        with (
            tc.tile_pool(name="phh", bufs=1) as phh,
            tc.tile_pool(name="phdw", bufs=1) as phdw,
            tc.tile_pool(name="psDt", bufs=3, space="PSUM") as psDt,
            tc.tile_pool(name="psDf", bufs=2, space="PSUM") as psDf,
        ):
            f1w_sb = phdw.tile([P, CCH // 2, 2, EXPAND_C], dt8)
            nc.gpsimd.dma_start(out=f1w_sb, in_=f1w_d)
            f2w_sb = phdw.tile([P, ECH // 2, 2, OUT_C], dt8)
            nc.gpsimd.dma_start(out=f2w_sb, in_=f2w_d)

            def emit_l2T(qt, act_only=False):
                # act_only: the DVE is busy with the attention flush when
                # qt0's transposes run, while the ACT queue is idle there
                l2T = phh.tile([P, CCH // 2, 2, 512], dt8, tag=f"l2T{qt}")
                for tc4 in range(4):
                    tci = qt * 4 + tc4
                    for fc in range(CCH):
                        pt2 = psDt.tile([P, P], dt_op)
                        nc.tensor.transpose(pt2, nrmA[:, tci, ts(fc, P)], ident)
                        dst = l2T[:, fc // 2, fc % 2, ts(tc4, P)]
                        if act_only or fc % 2:
                            nc.scalar.copy(out=dst, in_=pt2)
                        else:
                            nc.vector.tensor_copy(out=dst, in_=pt2)
                return l2T

            def emit_ff1(qt, l2T):
                hT = phh.tile([P, ECH // 2, 2, 512], dt8, tag="hT")
                for mc in range(ECH):
                    pf = psDf.tile([P, 512], f32, tag="pf")
                    for kc in range(CCH // 2):
                        nc.tensor.matmul(
                            pf, f1w_sb[:, kc, :, ts(mc, P)], l2T[:, kc, :, :],
                            start=(kc == 0), stop=(kc == CCH // 2 - 1),
                            perf_mode=mybir.MatmulPerfMode.DoubleRow,
                        )
                    nc.scalar.activation(
                        out=hT[:, mc // 2, mc % 2, :], in_=pf, func=AF.Gelu,
                        bias=f1b_sb[:, mc : mc + 1], scale=1.0 / F1_SCALE,
                    )
                return hT

            def emit_ff2(qt, hT):
                for tc4 in range(4):
                    tci = qt * 4 + tc4
                    xb = phd.tile([P, OUT_C], f32, tag="xb")
                    nc.vector.tensor_add(xb, X2[:, tci, :], f2b_bc)
                    pg = psDf.tile([P, OUT_C], f32, tag="pf")
                    for kc in range(ECH // 2):
                        nc.tensor.matmul(
                            pg, hT[:, kc, :, ts(tc4, P)], f2w_sb[:, kc, :, :],
                            start=(kc == 0), stop=(kc == ECH // 2 - 1),
                            perf_mode=mybir.MatmulPerfMode.DoubleRow,
                        )
                    outt = phd.tile([P, OUT_C], f32, tag="outt")
                    nc.vector.scalar_tensor_tensor(
                        out=outt, in0=pg, scalar=s2i_t, in1=xb,
                        op0=ALU.mult, op1=ALU.add,
                    )
                    nc.sync.dma_start(out=out_d[ts(tci, P), :], in_=outt)

            l2T0 = emit_l2T(0, act_only=True)
            emit_ln2(1, phd, stats_done=True)
            hT0 = emit_ff1(0, l2T0)
            l2T1 = emit_l2T(1)
            emit_ff2(0, hT0)
            hT1 = emit_ff1(1, l2T1)
            emit_ff2(1, hT1)

    persist.release()
    const.release()


def build_program(quant=QUANT):
    import concourse.bass as bass
    import concourse.tile as tile
    from concourse import mybir, bacc

    f32 = mybir.dt.float32
    dt_op = mybir.dt.bfloat16
    dt_w = dt_op

    nc = bacc.Bacc(None, target_bir_lowering=False)
    tens = {}

    def inp(name, shape, dt):
        tens[name] = nc.dram_tensor(name, list(shape), dt, kind="ExternalInput")

    inp("x_roll", (NTOK, IN_C), dt_w)
    inp("wqk", (IN_C, 2 * IN_C), dt_w)
    inp("bqk", (2 * IN_C,), f32)
    inp("wm", (P, H, IN_C // (2 * P), 2, IN_C), mybir.dt.float8e4)
    inp("merge_b", (OUT_C,), f32)
    inp("t_vec", (TIME_C,), f32)
    inp("time_w", (TIME_C, TIME_C), f32)
    inp("time_b", (TIME_C,), f32)
    inp("ff1_w", (P, (IN_C + TIME_C) // (2 * P), 2, EXPAND_C),
        mybir.dt.float8e4)
    inp("ff1_b", (EXPAND_C,), f32)
    inp("ff2_w", (P, EXPAND_C // (2 * P), 2, OUT_C), mybir.dt.float8e4)
    inp("ff2_b", (OUT_C,), f32)
    tens["out"] = nc.dram_tensor("out", [NQ, OUT_C], f32, kind="ExternalOutput")

    with tile.TileContext(nc) as tc:
        _emit(nc, tc, tens, dt_op)
    nc.finalize()
    return nc


def make_in_maps(x, t, ln1_g, ln1_b, qkv_w, qkv_b, merge_w, merge_b, time_w,
                 time_b, ln2_g, ln2_b, ff1_w, ff1_b, ff2_w, ff2_b, quant=QUANT):
    import ml_dtypes

    f = np.float32
    npdt = ml_dtypes.bfloat16

    x = np.asarray(x, f)
    qkv_w = np.asarray(qkv_w, f)
    qkv_b = np.asarray(qkv_b, f)
    # fold ln1 affine into qkv
    qkv_wf = np.asarray(ln1_g, f)[:, None] * qkv_w
    qkv_bf = qkv_b + np.asarray(ln1_b, f) @ qkv_w
    scale = HEAD_C**-0.5
    qkv_wf[:, : H * HEAD_C] *= scale
    qkv_bf[: H * HEAD_C] *= scale
    wqk = qkv_wf[:, : 2 * H * HEAD_C]
    bqk = qkv_bf[: 2 * H * HEAD_C]
    wv = qkv_wf[:, 2 * H * HEAD_C :]
    bv = qkv_bf[2 * H * HEAD_C :]
    merge_w = np.asarray(merge_w, f)
    # fold v bias into merge_b (softmax rows sum to 1)
    merge_bf = np.asarray(merge_b, f) + bv @ merge_w
    # fold merge_w into wv per head (scaled into fp8's normal range; the
    # 1/S_SCALE rides on the softmax reciprocal via the sca_t matmul rhs)
    wm = np.empty((IN_C, H * IN_C), f)
    for h in range(H):
        wm[:, h * IN_C : (h + 1) * IN_C] = (
            wv[:, h * IN_C : (h + 1) * IN_C]
            @ merge_w[h * IN_C : (h + 1) * IN_C, :]
        ) * S_SCALE
    # pack to the device layout [p, h, c, j, m]: row index = c*256+j*128+p,
    # so each head's slab is one contiguous 2KB line per partition
    wm = np.ascontiguousarray(
        wm.reshape(2, 2, P, H, IN_C).transpose(2, 3, 0, 1, 4)
    )
    # fold ln2 affine into ff1; scale + pack for fp8 DoubleRow
    # (row index = cc2*256 + jj*128 + p -> [p, cc2, jj, m], contiguous DMA)
    ff1_wf = np.asarray(ln2_g, f)[:, None] * np.asarray(ff1_w, f)
    ff1_bf = np.asarray(ff1_b, f) + np.asarray(ln2_b, f) @ np.asarray(ff1_w, f)
    ff1_wf = np.ascontiguousarray(
        (ff1_wf * F1_SCALE).reshape(3, 2, P, EXPAND_C).transpose(2, 0, 1, 3)
    )

    shared = {
        "wqk": wqk.astype(npdt),
        "bqk": bqk.astype(f),
        "wm": wm.astype(ml_dtypes.float8_e4m3),
        "merge_b": merge_bf.astype(f),
        "time_w": np.asarray(time_w, f),
        "time_b": np.asarray(time_b, f),
        "ff1_w": ff1_wf.astype(ml_dtypes.float8_e4m3),
        "ff1_b": ff1_bf.astype(f),
        "ff2_w": np.ascontiguousarray(
            (np.asarray(ff2_w, f) * F2_SCALE)
            .reshape(8, 2, P, OUT_C).transpose(2, 0, 1, 3)
        ).astype(ml_dtypes.float8_e4m3),
        "ff2_b": np.asarray(ff2_b, f),
    }
    in_maps = []
    for c in range(NCORES):
        b, half = divmod(c, 2)
        xb = x[b]
        x_roll = np.concatenate([xb[half * NQ :], xb[: half * NQ]], axis=0)
        m = dict(shared)
        m["x_roll"] = np.ascontiguousarray(x_roll).astype(npdt)
        m["t_vec"] = np.asarray(t, f)[b]
        in_maps.append(m)
    return in_maps


_CACHE = {}


def kernel(**inputs):
    from concourse.bass_utils import run_bass_kernel_spmd

    if "nc" not in _CACHE:
        _CACHE["nc"] = build_program(QUANT)
    nc = _CACHE["nc"]
    in_maps = make_in_maps(**inputs, quant=QUANT)
    res = run_bass_kernel_spmd(nc, in_maps, core_ids=list(range(NCORES)))
    out = np.stack([res.results[c]["out"] for c in range(NCORES)], axis=0)
    return out.reshape(B, N, OUT_C)
